# revision 1
# baseline (speedup 1.0000x reference)
"""Trainium2 Bass kernel for a 4-layer LSTM decoder step with Bahdanau attention.

Math (B=128 batch, S=128 enc positions, H=A=E_enc=1024, emb=64, V=32000, NL=4):
  x   = E[tokens]
  o1  = LSTM_f([x, context], hidden0, cell0)
  ad  = o1 @ Wad.T + bad ; scores[s,b] = (enc @ Wae.T + bae)[s,b,:] . ad[b,:]
  ctx = softmax_s(scores)-weighted sum of enc over s
  h   = LSTM_l0([o1, ctx]) -> LSTM_r1(h) -> LSTM_r2(h)
  out = [h, ctx] @ Wout.T + bout                               # [128, 32000]

Distribution over 8 NeuronCores:
  - LSTM layers: tensor-parallel over hidden dim (each core computes a 128-wide
    hidden shard = 512 of the 4096 gate rows); full h re-assembled with an
    AllGather after every layer.
  - Attention: sharded over encoder positions s (16 per core). scores use the
    identity  scores[s,b] = enc[s,b,:].(ad@Wae)[b,:] + ad[b,:].bae, so the
    [S,B,128] "ae" tensor is never materialized. Per-core partial
    exp-weighted context + partial sum(exp) are combined with one AllReduce;
    softmax normalization happens after (exp without max-subtraction is safe:
    scores are in [-10, 10] for this model scale).
  - Output projection: vocab-sharded (4000 rows of Wout per core, padded to
    4096); shards are concatenated on the host.

All activations on device live in transposed [feature, batch] layout so every
matmul contraction is on the partition axis; weights are pre-transposed and
pre-sharded on the host (weight-stationary kernels normally receive
pre-transposed weights).
"""
import os
import sys

sys.path.insert(0, "/opt/trn_rl_repo")

STAGE = int(os.environ.get("KERNEL_STAGE", "4"))

import numpy as np

from concourse import bacc, masks, mybir, tile
from concourse.bass_utils import run_bass_kernel_spmd

F32 = mybir.dt.float32
ALU = mybir.AluOpType
ACT = mybir.ActivationFunctionType

B = 128          # batch
S = 128          # encoder length
H = 1024         # hidden dim
NL = 4           # LSTM layers
KATT = 128       # attention projection size
E = 1024         # encoder hidden dim
NCORES = 8
HSH = H // NCORES        # 128: hidden shard per core
GSH = 4 * HSH            # 512: gate rows per core
SSH = S // NCORES        # 16: encoder positions per core
VSH = 32000 // NCORES    # 4000: vocab shard
VPAD = 4096              # padded vocab shard (8 x 512)
NV = VPAD // 512         # 8 vocab blocks of 512
XC = 1152                # padded [emb(64) + context(1024)] input width (9 x 128)

_compiled = None


def _build():
    nc = bacc.Bacc("TRN2", target_bir_lowering=False, debug=False,
                   num_devices=NCORES)

    def din(name, shape):
        return nc.dram_tensor(name, list(shape), F32, kind="ExternalInput").ap()

    xcT = din("xcT", [XC, B])                 # [emb+context, b] padded
    hT = din("hT", [NL, H, B])                # full prev hidden, transposed
    cT = din("cT", [NL, HSH, B])              # cell shard, transposed
    wih = [din(f"wih{l}", [(XC, H * 2, H, H)[l], GSH]) for l in range(NL)]
    whh = [din(f"whh{l}", [H, GSH]) for l in range(NL)]
    bias = [din(f"b{l}", [HSH, 4]) for l in range(NL)]
    wadT = din("wadT", [H, KATT])
    bad_c = din("bad", [KATT, 1])
    wae = din("wae", [KATT, E])
    bae_c = din("bae", [KATT, 1])
    enc = din("enc", [SSH, B, E])             # encoder outputs, s-shard
    wout = din("wout", [NV, 16, 128, 512])    # [vblock, kchunk, k, v]
    bout = din("bout", [1, VPAD])
    out = nc.dram_tensor("out", [B, VPAD], F32, kind="ExternalOutput").ap()

    rg = [list(range(NCORES))]

    with tile.TileContext(nc) as tc:
        with tc.tile_pool(name="const", bufs=1) as const, \
             tc.tile_pool(name="wstream", bufs=1) as wstream, \
             tc.tile_pool(name="acts", bufs=1) as acts, \
             tc.tile_pool(name="encp", bufs=1) as encp, \
             tc.tile_pool(name="scratch", bufs=1) as scratch, \
             tc.tile_pool(name="woutp", bufs=1) as woutp, \
             tc.tile_pool(name="gps", bufs=1, space="PSUM") as gps, \
             tc.tile_pool(name="outps", bufs=1, space="PSUM") as outps, \
             tc.tile_pool(name="trps", bufs=1, space="PSUM") as trps, \
             tc.tile_pool(name="dram", bufs=1, space="DRAM") as dram:

            # ---- constants ----
            ident = const.tile([128, 128], F32, tag="ident")
            masks.make_identity(nc, ident[:])
            ones = const.tile([1, 128], F32, tag="ones")
            nc.vector.memset(ones[:], 1.0)
            bias_sb = []
            for l in range(NL):
                t = const.tile([HSH, 4], F32, tag=f"bias{l}")
                nc.sync.dma_start(t[:], bias[l][:])
                bias_sb.append(t)
            bad_sb = const.tile([KATT, 1], F32, tag="bad")
            nc.sync.dma_start(bad_sb[:], bad_c[:])
            bae_sb = const.tile([KATT, 1], F32, tag="bae")
            nc.sync.dma_start(bae_sb[:], bae_c[:])
            wae_sb = const.tile([KATT, E], F32, tag="wae")
            nc.sync.dma_start(wae_sb[:], wae[:])
            bout_sb = const.tile([1, VPAD], F32, tag="bout", bufs=1, name="bout_sb")
            nc.sync.dma_start(bout_sb[:], bout[:])
            cT_sb = []
            for l in range(NL):
                t = const.tile([HSH, B], F32, tag=f"cT{l}")
                nc.sync.dma_start(t[:], cT[l])
                cT_sb.append(t)
            # full transposed prev-hidden per layer, as 8 [128, B] chunks
            hT_sb = []
            for l in range(NL):
                chunks = []
                for k in range(H // 128):
                    t = acts.tile([128, B], F32, tag="hTin", bufs=32, name="hTin")
                    nc.sync.dma_start(t[:], hT[l, k * 128:(k + 1) * 128, :])
                    chunks.append(t)
                hT_sb.append(chunks)
            # layer-f input [x, context] transposed, 9 chunks
            xcT_sb = []
            for k in range(XC // 128):
                t = acts.tile([128, B], F32, tag="xcT", bufs=9, name="xcT")
                nc.sync.dma_start(t[:], xcT[k * 128:(k + 1) * 128, :])
                xcT_sb.append(t)
            # encoder output slices (one per local s)
            enc_sb = []
            for s in range(SSH):
                t = encp.tile([B, E], F32, tag="enc", bufs=8, name="enc")
                nc.sync.dma_start(t[:], enc[s])
                enc_sb.append(t)

            # ---- one LSTM layer (gate rows sharded 8-way) ----
            def lstm_layer(l, xT_chunks):
                """xT_chunks: list of [128, B] SBUF tiles (layer input, transposed).
                Returns h-shard [HSH, B] tile."""
                wih_t, whh_t = [], []
                for k in range(len(xT_chunks)):
                    t = wstream.tile([128, GSH], F32, tag="wstream", bufs=10, name="wstream")
                    nc.sync.dma_start(t[:], wih[l][k * 128:(k + 1) * 128, :])
                    wih_t.append(t)
                for k in range(H // 128):
                    t = wstream.tile([128, GSH], F32, tag="wstream", bufs=10, name="wstream")
                    nc.sync.dma_start(t[:], whh[l][k * 128:(k + 1) * 128, :])
                    whh_t.append(t)
                ps = [gps.tile([HSH, B], F32, tag=f"gate{g}", bufs=1, name=f"gate{g}")
                      for g in range(4)]
                nk = len(xT_chunks) + H // 128
                ki = 0
                for k, xt in enumerate(xT_chunks):
                    for g in range(4):
                        nc.tensor.matmul(ps[g][:], wih_t[k][:, g * HSH:(g + 1) * HSH],
                                         xt[:], start=(ki == 0), stop=(ki == nk - 1))
                    ki += 1
                for k in range(H // 128):
                    for g in range(4):
                        nc.tensor.matmul(ps[g][:], whh_t[k][:, g * HSH:(g + 1) * HSH],
                                         hT_sb[l][k][:], start=(ki == 0), stop=(ki == nk - 1))
                    ki += 1
                sig_i = acts.tile([HSH, B], F32, tag="lstm_tmp", bufs=8, name="lstm_tmp")
                sig_f = acts.tile([HSH, B], F32, tag="lstm_tmp", bufs=8, name="lstm_tmp")
                tan_g = acts.tile([HSH, B], F32, tag="lstm_tmp", bufs=8, name="lstm_tmp")
                sig_o = acts.tile([HSH, B], F32, tag="lstm_tmp", bufs=8, name="lstm_tmp")
                nc.scalar.activation(sig_i[:], ps[0][:], ACT.Sigmoid, bias=bias_sb[l][:, 0:1])
                nc.scalar.activation(sig_f[:], ps[1][:], ACT.Sigmoid, bias=bias_sb[l][:, 1:2])
                nc.scalar.activation(tan_g[:], ps[2][:], ACT.Tanh, bias=bias_sb[l][:, 2:3])
                nc.scalar.activation(sig_o[:], ps[3][:], ACT.Sigmoid, bias=bias_sb[l][:, 3:4])
                t1 = acts.tile([HSH, B], F32, tag="lstm_tmp", bufs=8, name="lstm_tmp")
                t2 = acts.tile([HSH, B], F32, tag="lstm_tmp", bufs=8, name="lstm_tmp")
                nc.vector.tensor_tensor(t1[:], sig_f[:], cT_sb[l][:], ALU.mult)
                nc.vector.tensor_tensor(t2[:], sig_i[:], tan_g[:], ALU.mult)
                c2 = acts.tile([HSH, B], F32, tag="lstm_tmp", bufs=8, name="lstm_tmp")
                nc.vector.tensor_tensor(c2[:], t1[:], t2[:], ALU.add)
                tc2 = acts.tile([HSH, B], F32, tag="lstm_tmp", bufs=8, name="lstm_tmp")
                nc.scalar.activation(tc2[:], c2[:], ACT.Tanh)
                h = acts.tile([HSH, B], F32, tag="lstm_h", bufs=2, name="lstm_h")
                nc.vector.tensor_tensor(h[:], sig_o[:], tc2[:], ALU.mult)
                return h

            def allgather_h(h_tile, name):
                """h-shard [HSH, B] -> 8 chunks [128, B] of the full hT."""
                cc_in = dram.tile([HSH, B], F32, tag=f"agi_{name}")
                cc_out = dram.tile([H, B], F32, tag=f"ago_{name}")
                nc.sync.dma_start(cc_in[:], h_tile[:])
                nc.gpsimd.collective_compute(
                    "AllGather", ALU.bypass, replica_groups=rg,
                    ins=[cc_in[:].opt()], outs=[cc_out[:].opt()])
                chunks = []
                for k in range(H // 128):
                    t = acts.tile([128, B], F32, tag="hg_" + name, bufs=8, name="hgather")
                    nc.sync.dma_start(t[:], cc_out[k * 128:(k + 1) * 128, :])
                    chunks.append(t)
                return chunks

            # ---- layer f + allgather o1 ----
            h1 = lstm_layer(0, xcT_sb)
            o1T = allgather_h(h1, "h1")
            if STAGE == 1:
                for k in range(8):
                    nc.sync.dma_start(out[:, k * 128:(k + 1) * 128], o1T[k][:])

            # ---- attention ----
            if STAGE >= 2:
                # adT[kk, b] = Wad @ o1T + bad
                ad_ps = trps.tile([KATT, B], F32, tag="tr", bufs=2, name="ad_ps")
                for k in range(H // 128):
                    wt = wstream.tile([128, KATT], F32, tag="wstream_s", bufs=4, name="wstream_s")
                    nc.sync.dma_start(wt[:], wadT[k * 128:(k + 1) * 128, :])
                    nc.tensor.matmul(ad_ps[:], wt[:], o1T[k][:],
                                     start=(k == 0), stop=(k == H // 128 - 1))
                adT_sb = acts.tile([KATT, B], F32, tag="adT")
                nc.scalar.activation(adT_sb[:], ad_ps[:], ACT.Identity, bias=bad_sb[:])
                # w[b, e] = ad @ Wae ; cdot[b] = ad . bae
                w_sb = acts.tile([B, E], F32, tag="w_att")
                for half in range(2):
                    wps = outps.tile([B, 512], F32, tag="outps", bufs=2, name="wps")
                    nc.tensor.matmul(wps[:], adT_sb[:], wae_sb[:, half * 512:(half + 1) * 512],
                                     start=True, stop=True)
                    nc.vector.tensor_copy(w_sb[:, half * 512:(half + 1) * 512], wps[:])
                c_ps = trps.tile([B, 1], F32, tag="tr", bufs=2, name="c_ps")
                nc.tensor.matmul(c_ps[:], adT_sb[:], bae_sb[:], start=True, stop=True)
                cdot = acts.tile([B, 1], F32, tag="cdot")
                nc.vector.tensor_copy(cdot[:], c_ps[:])
                # per local s: scores -> exp -> weighted accumulation of enc
                alphas = acts.tile([B, SSH], F32, tag="alphas")
                scoresb = acts.tile([B, SSH], F32, tag="scoresb")
                ctx_acc = acts.tile([B, E], F32, tag="ctx_acc")
                for s in range(SSH):
                    prod = scratch.tile([B, E], F32, tag="prod", bufs=2, name="prod")
                    nc.vector.tensor_tensor(prod[:], enc_sb[s][:], w_sb[:], ALU.mult)
                    nc.vector.tensor_reduce(scoresb[:, s:s + 1], prod[:],
                                            mybir.AxisListType.X, ALU.add)
                    nc.scalar.activation(alphas[:, s:s + 1], scoresb[:, s:s + 1],
                                         ACT.Exp, bias=cdot[:])
                    if s == 0:
                        nc.scalar.activation(ctx_acc[:], enc_sb[s][:], ACT.Copy,
                                             scale=alphas[:, s:s + 1])
                    else:
                        wenc = scratch.tile([B, E], F32, tag="wenc", bufs=2, name="wenc")
                        nc.scalar.activation(wenc[:], enc_sb[s][:], ACT.Copy,
                                             scale=alphas[:, s:s + 1])
                        nc.vector.tensor_tensor(ctx_acc[:], ctx_acc[:], wenc[:], ALU.add)
                sumexp = acts.tile([B, 1], F32, tag="sumexp")
                nc.vector.tensor_reduce(sumexp[:], alphas[:], mybir.AxisListType.X, ALU.add)
                # AllReduce partial [ctx_acc | sumexp]
                ar_in = dram.tile([B, E + 8], F32, tag="ar_in")
                ar_out = dram.tile([B, E + 8], F32, tag="ar_out")
                nc.sync.dma_start(ar_in[:, 0:E], ctx_acc[:])
                se8 = acts.tile([B, 8], F32, tag="se8", bufs=1, name="se8")
                nc.vector.tensor_copy(se8[:], sumexp[:].to_broadcast([B, 8]))
                nc.sync.dma_start(ar_in[:, E:E + 8], se8[:])
                nc.gpsimd.collective_compute(
                    "AllReduce", ALU.add, replica_groups=rg,
                    ins=[ar_in[:].opt()], outs=[ar_out[:].opt()])
                ctx_raw = acts.tile([B, E], F32, tag="ctx_raw")
                nc.sync.dma_start(ctx_raw[:], ar_out[:, 0:E])
                se_sb = acts.tile([B, 1], F32, tag="se")
                nc.sync.dma_start(se_sb[:], ar_out[:, E:E + 1])
                recip = acts.tile([B, 1], F32, tag="recip")
                nc.vector.reciprocal(recip[:], se_sb[:])
                ctx_sb = acts.tile([B, E], F32, tag="ctx_sb")
                nc.scalar.activation(ctx_sb[:], ctx_raw[:], ACT.Copy, scale=recip[:])
                # transpose ctx -> 8 chunks [128, B]
                ctxT = []
                for k in range(E // 128):
                    tp = trps.tile([128, B], F32, tag="tr", bufs=2, name="tp")
                    nc.tensor.transpose(tp[:], ctx_sb[:, k * 128:(k + 1) * 128], ident[:])
                    t = acts.tile([128, B], F32, tag="ctxT", bufs=8, name="ctxT")
                    nc.vector.tensor_copy(t[:], tp[:])
                    ctxT.append(t)

            if STAGE == 2:
                nc.sync.dma_start(out[:, 0:E], ctx_sb[:])

            if STAGE >= 3:
                # ---- layers l0, r1, r2 ----
                h2 = lstm_layer(1, o1T + ctxT)
                h2T = allgather_h(h2, "h2")
                h3 = lstm_layer(2, h2T)
                h3T = allgather_h(h3, "h3")
                h4 = lstm_layer(3, h3T)
                h4T = allgather_h(h4, "h4")

            if STAGE == 3:
                for k in range(8):
                    nc.sync.dma_start(out[:, k * 128:(k + 1) * 128], h4T[k][:])

            if STAGE >= 4:
                # ---- output projection: out[b, v] = [h, ctx] @ Wout.T + bout ----
                # Split per vocab block: bias + ctx-half (k-chunks 8..15) can
                # start as soon as ctxT exists, streaming half of Wout during
                # the remaining LSTM layers; only the h-half waits for h4T.
                parts = []
                for vb in range(NV):
                    ps = outps.tile([B, 512], F32, tag="outps", bufs=2, name="ps")
                    nc.tensor.matmul(ps[:], ones[:], bout_sb[:, vb * 512:(vb + 1) * 512],
                                     start=True, stop=False)
                    for kc in range(8, 16):
                        wt = woutp.tile([128, 512], F32, tag="wout", bufs=16, name="wout")
                        nc.sync.dma_start(wt[:], wout[vb, kc])
                        nc.tensor.matmul(ps[:], ctxT[kc - 8][:], wt[:],
                                         start=False, stop=(kc == 15))
                    pt = acts.tile([B, 512], F32, tag="outpart", bufs=8, name="outpart")
                    nc.vector.tensor_copy(pt[:], ps[:])
                    parts.append(pt)
                for vb in range(NV):
                    ps = outps.tile([B, 512], F32, tag="outps", bufs=2, name="ps")
                    for kc in range(8):
                        wt = woutp.tile([128, 512], F32, tag="wout", bufs=16, name="wout")
                        nc.sync.dma_start(wt[:], wout[vb, kc])
                        nc.tensor.matmul(ps[:], h4T[kc][:], wt[:],
                                         start=(kc == 0), stop=(kc == 7))
                    ot = scratch.tile([B, 512], F32, tag="outsb", bufs=2, name="outsb")
                    nc.vector.tensor_tensor(ot[:], ps[:], parts[vb][:], ALU.add)
                    nc.sync.dma_start(out[:, vb * 512:(vb + 1) * 512], ot[:])

    nc.compile()
    return nc


def _prep_in_maps(inputs):
    f32 = lambda a: np.ascontiguousarray(np.asarray(a), dtype=np.float32)
    tokens = np.asarray(inputs["tokens"]).astype(np.int64)
    Emb = f32(inputs["E"])
    context = f32(inputs["context"])
    hidden = f32(inputs["hidden"])
    cell = f32(inputs["cell"])
    enc_out = f32(inputs["enc_outputs"])

    x = Emb[tokens]                                        # [B, 64]
    xc = np.concatenate([x, context], axis=1)              # [B, 1088]
    xc = np.pad(xc, ((0, 0), (0, XC - xc.shape[1])))       # [B, 1152]
    xcT = np.ascontiguousarray(xc.T)                       # [1152, B]
    hT = np.ascontiguousarray(hidden.transpose(0, 2, 1))   # [NL, H, B]

    wih_full = [f32(inputs["W_ih_f"]), f32(inputs["W_ih_l0"]),
                f32(inputs["W_ih_rest"])[0], f32(inputs["W_ih_rest"])[1]]
    whh_full = [f32(inputs["W_hh_f"]), f32(inputs["W_hh_l0"]),
                f32(inputs["W_hh_rest"])[0], f32(inputs["W_hh_rest"])[1]]
    b_full = [f32(inputs["b_ih_f"]) + f32(inputs["b_hh_f"]),
              f32(inputs["b_ih_l0"]) + f32(inputs["b_hh_l0"]),
              f32(inputs["b_ih_rest"])[0] + f32(inputs["b_hh_rest"])[0],
              f32(inputs["b_ih_rest"])[1] + f32(inputs["b_hh_rest"])[1]]

    wadT = np.ascontiguousarray(f32(inputs["Wad"]).T)      # [H, 128]
    bad_c = f32(inputs["bad"]).reshape(KATT, 1)
    wae = f32(inputs["Wae"])                               # [128, E]
    bae_c = f32(inputs["bae"]).reshape(KATT, 1)
    Wout = f32(inputs["Wout"])
    bout_full = f32(inputs["bout"])

    def gate_shard(W, c):
        # [4096, in] -> [in, 512]: rows for gates i,f,g,o of hidden dims
        # c*128:(c+1)*128, transposed.
        rows = np.concatenate(
            [W[g * H + c * HSH: g * H + (c + 1) * HSH] for g in range(4)], axis=0)
        return np.ascontiguousarray(rows.T)

    in_maps = []
    for c in range(NCORES):
        m = {"xcT": xcT, "hT": hT,
             "cT": np.ascontiguousarray(
                 cell[:, :, c * HSH:(c + 1) * HSH].transpose(0, 2, 1)),
             "wadT": wadT, "bad": bad_c, "wae": wae, "bae": bae_c,
             "enc": enc_out[c * SSH:(c + 1) * SSH],
             "bout": np.pad(bout_full[c * VSH:(c + 1) * VSH],
                            (0, VPAD - VSH)).reshape(1, VPAD)}
        for l in range(NL):
            wt = gate_shard(wih_full[l], c)
            if l == 0:
                wt = np.pad(wt, ((0, XC - wt.shape[0]), (0, 0)))
            m[f"wih{l}"] = wt
            m[f"whh{l}"] = gate_shard(whh_full[l], c)
            b = b_full[l]
            bsh = np.concatenate(
                [b[g * H + c * HSH: g * H + (c + 1) * HSH] for g in range(4)])
            m[f"b{l}"] = np.ascontiguousarray(bsh.reshape(4, HSH).T)
        Wsh = Wout[c * VSH:(c + 1) * VSH]                   # [4000, 2048]
        Wsh = np.pad(Wsh, ((0, VPAD - VSH), (0, 0)))        # [4096, 2048]
        WT = Wsh.T                                          # [2048, 4096]
        m["wout"] = np.ascontiguousarray(
            WT.reshape(16, 128, NV, 512).transpose(2, 0, 1, 3))
        in_maps.append(m)
    return in_maps


def get_compiled():
    global _compiled
    if _compiled is None:
        _compiled = _build()
    return _compiled


def kernel(**inputs):
    nc = get_compiled()
    in_maps = _prep_in_maps(inputs)
    res = run_bass_kernel_spmd(nc, in_maps, core_ids=list(range(NCORES)))
    out = np.concatenate([res.results[c]["out"][:, :VSH] for c in range(NCORES)],
                         axis=1)
    return out



# revision 4
# speedup vs baseline: 1.6338x; 1.6338x over previous
"""Trainium2 Bass kernel for a 4-layer LSTM decoder step with Bahdanau attention.

Math (B=128 batch, S=128 enc positions, H=A=E_enc=1024, emb=64, V=32000, NL=4):
  x   = E[tokens]
  o1  = LSTM_f([x, context], hidden0, cell0)
  ad  = o1 @ Wad.T + bad ; scores[s,b] = (enc @ Wae.T + bae)[s,b,:] . ad[b,:]
  ctx = softmax_s(scores)-weighted sum of enc over s
  h   = LSTM_l0([o1, ctx]) -> LSTM_r1(h) -> LSTM_r2(h)
  out = [h, ctx] @ Wout.T + bout                               # [128, 32000]

Distribution over 8 NeuronCores:
  - LSTM layers: tensor-parallel over hidden dim (each core computes a 128-wide
    hidden shard = 512 of the 4096 gate rows); full h re-assembled with an
    AllGather after every layer.
  - Attention: sharded over encoder positions s (16 per core). scores use the
    identity  scores[s,b] = enc[s,b,:].(ad@Wae)[b,:] + ad[b,:].bae, so the
    [S,B,128] "ae" tensor is never materialized. Per-core partial
    exp-weighted context + partial sum(exp) are combined with one AllReduce;
    softmax normalization happens after (exp without max-subtraction is safe:
    scores are in [-10, 10] for this model scale).
  - Output projection: vocab-sharded (4000 rows of Wout per core, padded to
    4096); shards are concatenated on the host.

All activations on device live in transposed [feature, batch] layout so every
matmul contraction is on the partition axis; weights are pre-transposed and
pre-sharded on the host (weight-stationary kernels normally receive
pre-transposed weights).
"""
import os
import sys

sys.path.insert(0, "/opt/trn_rl_repo")

STAGE = int(os.environ.get("KERNEL_STAGE", "4"))
NOCC = bool(int(os.environ.get("KERNEL_NOCC", "0")))  # timing-sim stand-in mode

import numpy as np

from concourse import bacc, masks, mybir, tile
from concourse.bass_utils import run_bass_kernel_spmd

F32 = mybir.dt.float32
ALU = mybir.AluOpType
ACT = mybir.ActivationFunctionType

B = 128          # batch
S = 128          # encoder length
H = 1024         # hidden dim
NL = 4           # LSTM layers
KATT = 128       # attention projection size
E = 1024         # encoder hidden dim
NCORES = 8
HSH = H // NCORES        # 128: hidden shard per core
GSH = 4 * HSH            # 512: gate rows per core
SSH = S // NCORES        # 16: encoder positions per core
VSH = 32000 // NCORES    # 4000: vocab shard
VPAD = 4096              # padded vocab shard (8 x 512)
NV = VPAD // 512         # 8 vocab blocks of 512
XC = 1152                # padded [emb(64) + context(1024)] input width (9 x 128)

_compiled = None


def _build():
    nc = bacc.Bacc("TRN2", target_bir_lowering=False, debug=False,
                   num_devices=NCORES)

    def din(name, shape):
        return nc.dram_tensor(name, list(shape), F32, kind="ExternalInput").ap()

    xcT = din("xcT", [XC, B])                 # [emb+context, b] padded
    hT = din("hT", [NL, H, B])                # full prev hidden, transposed
    cT = din("cT", [NL, HSH, B])              # cell shard, transposed
    wih = [din(f"wih{l}", [(XC, H * 2, H, H)[l], GSH]) for l in range(NL)]
    whh = [din(f"whh{l}", [H, GSH]) for l in range(NL)]
    bias = [din(f"b{l}", [HSH, 4]) for l in range(NL)]
    wadT = din("wadT", [H, KATT])
    bad_c = din("bad", [KATT, 1])
    wae = din("wae", [KATT, E])
    bae_c = din("bae", [KATT, 1])
    enc = din("enc", [SSH, B, E])             # encoder outputs, s-shard
    wout = din("wout", [NV, 16, 128, 512])    # [vblock, kchunk, k, v]
    bout = din("bout", [1, VPAD])
    out = nc.dram_tensor("out", [B, VPAD], F32, kind="ExternalOutput").ap()

    rg = [list(range(NCORES))]

    with tile.TileContext(nc) as tc:
        with tc.tile_pool(name="const", bufs=1) as const, \
             tc.tile_pool(name="wstream", bufs=1) as wstream, \
             tc.tile_pool(name="acts", bufs=1) as acts, \
             tc.tile_pool(name="encp", bufs=1) as encp, \
             tc.tile_pool(name="scratch", bufs=1) as scratch, \
             tc.tile_pool(name="woutp", bufs=1) as woutp, \
             tc.tile_pool(name="gps", bufs=1, space="PSUM") as gps, \
             tc.tile_pool(name="outps", bufs=1, space="PSUM") as outps, \
             tc.tile_pool(name="trps", bufs=1, space="PSUM") as trps, \
             tc.tile_pool(name="dram", bufs=1, space="DRAM") as dram:

            # ---- constants ----
            ident = const.tile([128, 128], F32, tag="ident")
            masks.make_identity(nc, ident[:])
            ones = const.tile([1, 128], F32, tag="ones")
            nc.vector.memset(ones[:], 1.0)
            bias_sb = []
            for l in range(NL):
                t = const.tile([HSH, 4], F32, tag=f"bias{l}")
                nc.sync.dma_start(t[:], bias[l][:])
                bias_sb.append(t)
            bad_sb = const.tile([KATT, 1], F32, tag="bad")
            nc.sync.dma_start(bad_sb[:], bad_c[:])
            bae_sb = const.tile([KATT, 1], F32, tag="bae")
            nc.sync.dma_start(bae_sb[:], bae_c[:])
            wae_sb = const.tile([KATT, E], F32, tag="wae")
            nc.sync.dma_start(wae_sb[:], wae[:])
            bout_sb = const.tile([1, VPAD], F32, tag="bout", bufs=1, name="bout_sb")
            nc.sync.dma_start(bout_sb[:], bout[:])
            cT_sb = []
            for l in range(NL):
                t = const.tile([HSH, B], F32, tag=f"cT{l}")
                nc.sync.dma_start(t[:], cT[l])
                cT_sb.append(t)
            # full transposed prev-hidden per layer, as 8 [128, B] chunks
            hT_sb = []
            for l in range(NL):
                chunks = []
                for k in range(H // 128):
                    t = acts.tile([128, B], F32, tag="hTin", bufs=32, name="hTin")
                    nc.sync.dma_start(t[:], hT[l, k * 128:(k + 1) * 128, :])
                    chunks.append(t)
                hT_sb.append(chunks)
            # layer-f input [x, context] transposed, 9 chunks
            xcT_sb = []
            for k in range(XC // 128):
                t = acts.tile([128, B], F32, tag="xcT", bufs=9, name="xcT")
                nc.sync.dma_start(t[:], xcT[k * 128:(k + 1) * 128, :])
                xcT_sb.append(t)
            # encoder output slices (one per local s)
            enc_sb = []
            for s in range(SSH):
                t = encp.tile([B, E], F32, tag="enc", bufs=8, name="enc")
                nc.sync.dma_start(t[:], enc[s])
                enc_sb.append(t)

            # ---- one LSTM layer (gate rows sharded 8-way) ----
            def lstm_layer(l, xT_chunks):
                """xT_chunks: list of [128, B] SBUF tiles (layer input, transposed).
                Returns h-shard [HSH, B] tile."""
                wih_t, whh_t = [], []
                for k in range(len(xT_chunks)):
                    t = wstream.tile([128, GSH], F32, tag="wstream", bufs=10, name="wstream")
                    nc.sync.dma_start(t[:], wih[l][k * 128:(k + 1) * 128, :])
                    wih_t.append(t)
                for k in range(H // 128):
                    t = wstream.tile([128, GSH], F32, tag="wstream", bufs=10, name="wstream")
                    nc.sync.dma_start(t[:], whh[l][k * 128:(k + 1) * 128, :])
                    whh_t.append(t)
                ps = [gps.tile([HSH, B], F32, tag=f"gate{g}", bufs=1, name=f"gate{g}")
                      for g in range(4)]
                nk = len(xT_chunks) + H // 128
                ki = 0
                for k, xt in enumerate(xT_chunks):
                    for g in range(4):
                        nc.tensor.matmul(ps[g][:], wih_t[k][:, g * HSH:(g + 1) * HSH],
                                         xt[:], start=(ki == 0), stop=(ki == nk - 1))
                    ki += 1
                for k in range(H // 128):
                    for g in range(4):
                        nc.tensor.matmul(ps[g][:], whh_t[k][:, g * HSH:(g + 1) * HSH],
                                         hT_sb[l][k][:], start=(ki == 0), stop=(ki == nk - 1))
                    ki += 1
                sig_i = acts.tile([HSH, B], F32, tag="lstm_tmp", bufs=8, name="lstm_tmp")
                sig_f = acts.tile([HSH, B], F32, tag="lstm_tmp", bufs=8, name="lstm_tmp")
                tan_g = acts.tile([HSH, B], F32, tag="lstm_tmp", bufs=8, name="lstm_tmp")
                sig_o = acts.tile([HSH, B], F32, tag="lstm_tmp", bufs=8, name="lstm_tmp")
                nc.scalar.activation(sig_i[:], ps[0][:], ACT.Sigmoid, bias=bias_sb[l][:, 0:1])
                nc.scalar.activation(sig_f[:], ps[1][:], ACT.Sigmoid, bias=bias_sb[l][:, 1:2])
                nc.scalar.activation(tan_g[:], ps[2][:], ACT.Tanh, bias=bias_sb[l][:, 2:3])
                nc.scalar.activation(sig_o[:], ps[3][:], ACT.Sigmoid, bias=bias_sb[l][:, 3:4])
                t1 = acts.tile([HSH, B], F32, tag="lstm_tmp", bufs=8, name="lstm_tmp")
                t2 = acts.tile([HSH, B], F32, tag="lstm_tmp", bufs=8, name="lstm_tmp")
                nc.vector.tensor_tensor(t1[:], sig_f[:], cT_sb[l][:], ALU.mult)
                nc.vector.tensor_tensor(t2[:], sig_i[:], tan_g[:], ALU.mult)
                c2 = acts.tile([HSH, B], F32, tag="lstm_tmp", bufs=8, name="lstm_tmp")
                nc.vector.tensor_tensor(c2[:], t1[:], t2[:], ALU.add)
                tc2 = acts.tile([HSH, B], F32, tag="lstm_tmp", bufs=8, name="lstm_tmp")
                nc.scalar.activation(tc2[:], c2[:], ACT.Tanh)
                h = acts.tile([HSH, B], F32, tag="lstm_h", bufs=2, name="lstm_h")
                nc.vector.tensor_tensor(h[:], sig_o[:], tc2[:], ALU.mult)
                return h

            def allgather_h(h_tile, name):
                """h-shard [HSH, B] -> 8 chunks [128, B] of the full hT."""
                cc_in = dram.tile([HSH, B], F32, tag=f"agi_{name}")
                cc_out = dram.tile([H, B], F32, tag=f"ago_{name}")
                nc.sync.dma_start(cc_in[:], h_tile[:])
                if NOCC:
                    for k in range(NCORES):
                        nc.sync.dma_start(cc_out[k * HSH:(k + 1) * HSH, :], cc_in[:])
                else:
                    nc.gpsimd.collective_compute(
                        "AllGather", ALU.bypass, replica_groups=rg,
                        ins=[cc_in[:].opt()], outs=[cc_out[:].opt()])
                chunks = []
                for k in range(H // 128):
                    t = acts.tile([128, B], F32, tag="hg_" + name, bufs=8, name="hgather")
                    nc.sync.dma_start(t[:], cc_out[k * 128:(k + 1) * 128, :])
                    chunks.append(t)
                return chunks

            # ---- layer f + allgather o1 ----
            h1 = lstm_layer(0, xcT_sb)
            o1T = allgather_h(h1, "h1")
            if STAGE == 1:
                for k in range(8):
                    nc.sync.dma_start(out[:, k * 128:(k + 1) * 128], o1T[k][:])

            # ---- attention ----
            if STAGE >= 2:
                # adT[kk, b] = Wad @ o1T + bad
                ad_ps = trps.tile([KATT, B], F32, tag="tr", bufs=2, name="ad_ps")
                for k in range(H // 128):
                    wt = wstream.tile([128, KATT], F32, tag="wstream_s", bufs=4, name="wstream_s")
                    nc.sync.dma_start(wt[:], wadT[k * 128:(k + 1) * 128, :])
                    nc.tensor.matmul(ad_ps[:], wt[:], o1T[k][:],
                                     start=(k == 0), stop=(k == H // 128 - 1))
                adT_sb = acts.tile([KATT, B], F32, tag="adT")
                nc.scalar.activation(adT_sb[:], ad_ps[:], ACT.Identity, bias=bad_sb[:])
                # w[b, e] = ad @ Wae ; cdot[b] = ad . bae
                w_sb = acts.tile([B, E], F32, tag="w_att")
                for half in range(2):
                    wps = outps.tile([B, 512], F32, tag="outps", bufs=2, name="wps")
                    nc.tensor.matmul(wps[:], adT_sb[:], wae_sb[:, half * 512:(half + 1) * 512],
                                     start=True, stop=True)
                    nc.vector.tensor_copy(w_sb[:, half * 512:(half + 1) * 512], wps[:])
                c_ps = trps.tile([B, 1], F32, tag="tr", bufs=2, name="c_ps")
                nc.tensor.matmul(c_ps[:], adT_sb[:], bae_sb[:], start=True, stop=True)
                cdot = acts.tile([B, 1], F32, tag="cdot")
                nc.vector.tensor_copy(cdot[:], c_ps[:])
                # per local s: scores -> exp -> weighted accumulation of enc
                alphas = acts.tile([B, SSH], F32, tag="alphas")
                scoresb = acts.tile([B, SSH], F32, tag="scoresb")
                ctx_acc = acts.tile([B, E], F32, tag="ctx_acc")
                for s in range(SSH):
                    prod = scratch.tile([B, E], F32, tag="prod", bufs=2, name="prod")
                    nc.vector.tensor_tensor(prod[:], enc_sb[s][:], w_sb[:], ALU.mult)
                    nc.vector.tensor_reduce(scoresb[:, s:s + 1], prod[:],
                                            mybir.AxisListType.X, ALU.add)
                    nc.scalar.activation(alphas[:, s:s + 1], scoresb[:, s:s + 1],
                                         ACT.Exp, bias=cdot[:])
                    if s == 0:
                        nc.scalar.activation(ctx_acc[:], enc_sb[s][:], ACT.Copy,
                                             scale=alphas[:, s:s + 1])
                    else:
                        wenc = scratch.tile([B, E], F32, tag="wenc", bufs=2, name="wenc")
                        nc.scalar.activation(wenc[:], enc_sb[s][:], ACT.Copy,
                                             scale=alphas[:, s:s + 1])
                        nc.vector.tensor_tensor(ctx_acc[:], ctx_acc[:], wenc[:], ALU.add)
                sumexp = acts.tile([B, 1], F32, tag="sumexp")
                nc.vector.tensor_reduce(sumexp[:], alphas[:], mybir.AxisListType.X, ALU.add)
                # AllReduce partial [ctx_acc | sumexp]
                ar_in = dram.tile([B, E + 8], F32, tag="ar_in")
                ar_out = dram.tile([B, E + 8], F32, tag="ar_out")
                nc.sync.dma_start(ar_in[:, 0:E], ctx_acc[:])
                se8 = acts.tile([B, 8], F32, tag="se8", bufs=1, name="se8")
                nc.vector.tensor_copy(se8[:], sumexp[:].to_broadcast([B, 8]))
                nc.sync.dma_start(ar_in[:, E:E + 8], se8[:])
                if NOCC:
                    nc.sync.dma_start(ar_out[:], ar_in[:])
                else:
                    nc.gpsimd.collective_compute(
                        "AllReduce", ALU.add, replica_groups=rg,
                        ins=[ar_in[:].opt()], outs=[ar_out[:].opt()])
                ctx_raw = acts.tile([B, E], F32, tag="ctx_raw")
                nc.sync.dma_start(ctx_raw[:], ar_out[:, 0:E])
                se_sb = acts.tile([B, 1], F32, tag="se")
                nc.sync.dma_start(se_sb[:], ar_out[:, E:E + 1])
                recip = acts.tile([B, 1], F32, tag="recip")
                nc.vector.reciprocal(recip[:], se_sb[:])
                ctx_sb = acts.tile([B, E], F32, tag="ctx_sb")
                nc.scalar.activation(ctx_sb[:], ctx_raw[:], ACT.Copy, scale=recip[:])
                # transpose ctx -> 8 chunks [128, B]
                ctxT = []
                for k in range(E // 128):
                    tp = trps.tile([128, B], F32, tag="tr", bufs=2, name="tp")
                    nc.tensor.transpose(tp[:], ctx_sb[:, k * 128:(k + 1) * 128], ident[:])
                    t = acts.tile([128, B], F32, tag="ctxT", bufs=8, name="ctxT")
                    nc.vector.tensor_copy(t[:], tp[:])
                    ctxT.append(t)

            if STAGE == 2:
                nc.sync.dma_start(out[:, 0:E], ctx_sb[:])

            if STAGE >= 3:
                # ---- layers l0, r1, r2 ----
                h2 = lstm_layer(1, o1T + ctxT)
                h2T = allgather_h(h2, "h2")
                h3 = lstm_layer(2, h2T)
                h3T = allgather_h(h3, "h3")
                h4 = lstm_layer(3, h3T)
                h4T = allgather_h(h4, "h4")

            if STAGE == 3:
                for k in range(8):
                    nc.sync.dma_start(out[:, k * 128:(k + 1) * 128], h4T[k][:])

            if STAGE >= 4:
                # ---- output projection: out[b, v] = [h, ctx] @ Wout.T + bout ----
                # Split per vocab block: bias + ctx-half (k-chunks 8..15) can
                # start as soon as ctxT exists, streaming half of Wout during
                # the remaining LSTM layers; only the h-half waits for h4T.
                parts = []
                for vb in range(NV):
                    ps = outps.tile([B, 512], F32, tag="outps", bufs=2, name="ps")
                    nc.tensor.matmul(ps[:], ones[:], bout_sb[:, vb * 512:(vb + 1) * 512],
                                     start=True, stop=False)
                    for kc in range(8, 16):
                        wt = woutp.tile([128, 512], F32, tag="wout", bufs=16, name="wout")
                        nc.sync.dma_start(wt[:], wout[vb, kc])
                        nc.tensor.matmul(ps[:], ctxT[kc - 8][:], wt[:],
                                         start=False, stop=(kc == 15))
                    pt = acts.tile([B, 512], F32, tag="outpart", bufs=8, name="outpart")
                    nc.vector.tensor_copy(pt[:], ps[:])
                    parts.append(pt)
                for vb in range(NV):
                    ps = outps.tile([B, 512], F32, tag="outps", bufs=2, name="ps")
                    for kc in range(8):
                        wt = woutp.tile([128, 512], F32, tag="wout", bufs=16, name="wout")
                        nc.sync.dma_start(wt[:], wout[vb, kc])
                        nc.tensor.matmul(ps[:], h4T[kc][:], wt[:],
                                         start=(kc == 0), stop=(kc == 7))
                    ot = scratch.tile([B, 512], F32, tag="outsb", bufs=2, name="outsb")
                    nc.vector.tensor_tensor(ot[:], ps[:], parts[vb][:], ALU.add)
                    nc.sync.dma_start(out[:, vb * 512:(vb + 1) * 512], ot[:])

    nc.compile()
    return nc


def _prep_in_maps(inputs):
    f32 = lambda a: np.ascontiguousarray(np.asarray(a), dtype=np.float32)
    tokens = np.asarray(inputs["tokens"]).astype(np.int64)
    Emb = f32(inputs["E"])
    context = f32(inputs["context"])
    hidden = f32(inputs["hidden"])
    cell = f32(inputs["cell"])
    enc_out = f32(inputs["enc_outputs"])

    x = Emb[tokens]                                        # [B, 64]
    xc = np.concatenate([x, context], axis=1)              # [B, 1088]
    xc = np.pad(xc, ((0, 0), (0, XC - xc.shape[1])))       # [B, 1152]
    xcT = np.ascontiguousarray(xc.T)                       # [1152, B]
    hT = np.ascontiguousarray(hidden.transpose(0, 2, 1))   # [NL, H, B]

    wih_full = [f32(inputs["W_ih_f"]), f32(inputs["W_ih_l0"]),
                f32(inputs["W_ih_rest"])[0], f32(inputs["W_ih_rest"])[1]]
    whh_full = [f32(inputs["W_hh_f"]), f32(inputs["W_hh_l0"]),
                f32(inputs["W_hh_rest"])[0], f32(inputs["W_hh_rest"])[1]]
    b_full = [f32(inputs["b_ih_f"]) + f32(inputs["b_hh_f"]),
              f32(inputs["b_ih_l0"]) + f32(inputs["b_hh_l0"]),
              f32(inputs["b_ih_rest"])[0] + f32(inputs["b_hh_rest"])[0],
              f32(inputs["b_ih_rest"])[1] + f32(inputs["b_hh_rest"])[1]]

    wadT = np.ascontiguousarray(f32(inputs["Wad"]).T)      # [H, 128]
    bad_c = f32(inputs["bad"]).reshape(KATT, 1)
    wae = f32(inputs["Wae"])                               # [128, E]
    bae_c = f32(inputs["bae"]).reshape(KATT, 1)
    Wout = f32(inputs["Wout"])
    bout_full = f32(inputs["bout"])

    def gate_shard(W, c):
        # [4096, in] -> [in, 512]: rows for gates i,f,g,o of hidden dims
        # c*128:(c+1)*128, transposed.
        rows = np.concatenate(
            [W[g * H + c * HSH: g * H + (c + 1) * HSH] for g in range(4)], axis=0)
        return np.ascontiguousarray(rows.T)

    in_maps = []
    for c in range(NCORES):
        m = {"xcT": xcT, "hT": hT,
             "cT": np.ascontiguousarray(
                 cell[:, :, c * HSH:(c + 1) * HSH].transpose(0, 2, 1)),
             "wadT": wadT, "bad": bad_c, "wae": wae, "bae": bae_c,
             "enc": enc_out[c * SSH:(c + 1) * SSH],
             "bout": np.pad(bout_full[c * VSH:(c + 1) * VSH],
                            (0, VPAD - VSH)).reshape(1, VPAD)}
        for l in range(NL):
            wt = gate_shard(wih_full[l], c)
            if l == 0:
                wt = np.pad(wt, ((0, XC - wt.shape[0]), (0, 0)))
            m[f"wih{l}"] = wt
            m[f"whh{l}"] = gate_shard(whh_full[l], c)
            b = b_full[l]
            bsh = np.concatenate(
                [b[g * H + c * HSH: g * H + (c + 1) * HSH] for g in range(4)])
            m[f"b{l}"] = np.ascontiguousarray(bsh.reshape(4, HSH).T)
        Wsh = Wout[c * VSH:(c + 1) * VSH]                   # [4000, 2048]
        Wsh = np.pad(Wsh, ((0, VPAD - VSH), (0, 0)))        # [4096, 2048]
        WT = Wsh.T                                          # [2048, 4096]
        m["wout"] = np.ascontiguousarray(
            WT.reshape(16, 128, NV, 512).transpose(2, 0, 1, 3))
        in_maps.append(m)
    return in_maps


def get_compiled():
    global _compiled
    if _compiled is None:
        _compiled = _build()
    return _compiled


def kernel(**inputs):
    nc = get_compiled()
    in_maps = _prep_in_maps(inputs)
    res = run_bass_kernel_spmd(nc, in_maps, core_ids=list(range(NCORES)))
    out = np.concatenate([res.results[c]["out"][:, :VSH] for c in range(NCORES)],
                         axis=1)
    return out

